# revision 9
# baseline (speedup 1.0000x reference)
"""AttentionPooling segment-reduce kernel for 8 Trainium2 NeuronCores.

Math (reference):
    k = x @ key_w.T + key_b            # [N, 256] -> heads [N, 4, 64]
    v = x @ value_w.T + value_b
    attn   = einsum('hd,nhd->nh', query, k) * SCALE
    w      = exp(attn)
    wsum   = segment_sum(w)[batch]
    out[b] = segment_sum(w/(wsum+EPS) * v)

Algebraic restructuring (exact):
    attn[n,h] = qt[:,h] . x[n] + sc[h],  qt = SCALE*(key_w^T q per head),
                                         sc = SCALE*(q . key_b per head)
    w = exp(attn) = g[h]*wt[n,h],  wt = exp(qt . x),  g = exp(sc)
    v' = x @ value_w.T                 (bias deferred to segment level)
    St[b,f] = sum_{n in b} wt[n,h(f)] v'[n,f];  dt[b,h] = sum_{n in b} wt[n,h]
    out[b,f] = (St[b,f] + dt[b,h]*value_b[f]) / (dt[b,h] + EPS/g[h])

Device mapping: core c owns segments [c*512,(c+1)*512) split into 4 windows of
128 segments; window nodes padded to 128-multiples. Per 128-node tile:
 - PE: fused projection psum[nodes,260] = xT_tile.T @ [Wv^T | qt] (fp16 in,
   fp32 accum), then segment reduce psum_s[segs,260] += onehot.T @ u.
 - ACT: exp of the 4 attn columns (batched over a 3-tile group).
 - DVE: u[:,0:256] = psum[:,0:256] * wt (head-broadcast), one batched op/group.
One-hot node->segment matrices are precomputed on the host (exact 0/1 fp16)
and streamed alongside x^T, so no on-device index compute is needed.
Window epilogue (DVE): out = (St + dt*bv) / (dt + eps/g), DMA to the core's
output rows. Host pre-transposes x to [256, N] fp16 so the contraction dim
lands on SBUF partitions.
"""

from contextlib import ExitStack

import numpy as np

N = 262144
DIM = 256
H = 4
HD = 64
B = 4096
SCALE = HD ** (-0.5)
EPS = 1e-8

NCORES = 8
SEGS_PER_CORE = B // NCORES          # 512
WPC = 4                              # windows per core
WSEG = SEGS_PER_CORE // WPC          # 128 segments per window
GRP = 3                              # node-tiles per PSUM group
CHUNK = 1024                         # x columns per DMA chunk

TRACE = False                        # test harness can flip for profiling
LAST_RESULT = None

_cache = {}


def _build(tw: int):
    """Build + compile the SPMD program for tw node-tiles per window."""
    import concourse.tile as tile
    from concourse import bacc, mybir

    F32 = mybir.dt.float32
    F16 = mybir.dt.float16
    Alu = mybir.AluOpType
    Act = mybir.ActivationFunctionType

    P = WPC * tw * 128

    nc = bacc.Bacc("TRN2", target_bir_lowering=False, debug=False,
                   num_devices=NCORES)

    xT_d = nc.dram_tensor("xT", [256, P], F16, kind="ExternalInput").ap()
    oh_d = nc.dram_tensor("oh", [128, P], F16, kind="ExternalInput").ap()
    wq_d = nc.dram_tensor("wq", [256, 260], F16, kind="ExternalInput").ap()
    bvrep_d = nc.dram_tensor("bvrep", [128, 256], F32,
                             kind="ExternalInput").ap()
    epsg_d = nc.dram_tensor("epsg", [128, 4], F32, kind="ExternalInput").ap()
    out_d = nc.dram_tensor("out", [SEGS_PER_CORE, 256], F32,
                           kind="ExternalOutput").ap()

    with tile.TileContext(nc) as tc, ExitStack() as ctx:
        consts = ctx.enter_context(tc.tile_pool(name="consts", bufs=1))
        xin = ctx.enter_context(tc.tile_pool(name="xin", bufs=6))
        up = ctx.enter_context(tc.tile_pool(name="up", bufs=4))
        fxp = ctx.enter_context(tc.tile_pool(name="fxp", bufs=2))
        pp = ctx.enter_context(tc.tile_pool(name="pp", bufs=2, space="PSUM"))
        sp = ctx.enter_context(tc.tile_pool(name="sp", bufs=2, space="PSUM"))

        wq0 = consts.tile([128, 260], F16, tag="wq0")
        wq1 = consts.tile([128, 260], F16, tag="wq1")
        bvrep = consts.tile([128, 256], F32, tag="bvrep")
        epsg = consts.tile([128, 4], F32, tag="epsg")
        nc.sync.dma_start(wq0[:], wq_d[0:128, :])
        nc.sync.dma_start(wq1[:], wq_d[128:256, :])
        nc.sync.dma_start(bvrep[:], bvrep_d)
        nc.sync.dma_start(epsg[:], epsg_d)

        xt0 = xt1 = ohc = None
        for w in range(WPC):
            psum_s = sp.tile([128, 260], F32, tag="ps")
            for g0 in range(0, tw, GRP):
                gsz = min(GRP, tw - g0)
                psum4 = pp.tile([128, gsz * 512], F32, tag="pp")
                u4 = up.tile([128, gsz * 260], F16, tag="u4")
                ohview = []
                for b in range(gsz):
                    t = w * tw + g0 + b          # core-local tile index
                    col = t * 128
                    if col % CHUNK == 0:
                        cw = min(CHUNK, P - col)
                        xt0 = xin.tile([128, CHUNK], F16, tag="xt0")
                        xt1 = xin.tile([128, CHUNK], F16, tag="xt1")
                        ohc = xin.tile([128, CHUNK], F16, tag="ohc")
                        nc.sync.dma_start(xt0[:, 0:cw],
                                          xT_d[0:128, col:col + cw])
                        nc.sync.dma_start(xt1[:, 0:cw],
                                          xT_d[128:256, col:col + cw])
                        nc.sync.dma_start(ohc[:, 0:cw],
                                          oh_d[:, col:col + cw])
                    o = col % CHUNK
                    ps = psum4[:, b * 512:b * 512 + 260]
                    nc.tensor.matmul(ps, xt0[:, o:o + 128], wq0[:],
                                     start=True, stop=False)
                    nc.tensor.matmul(ps, xt1[:, o:o + 128], wq1[:],
                                     start=False, stop=True)
                    ohview.append(ohc[:, o:o + 128])

                p3 = psum4[:].rearrange("p (b c) -> p b c", c=512)
                u3 = u4[:].rearrange("p (b c) -> p b c", c=260)
                nc.scalar.activation(u3[:, :, 256:260], p3[:, :, 256:260],
                                     Act.Exp)
                in0 = p3[:, :, 0:256].rearrange("p b (h d) -> p b h d", h=H)
                in1 = (u3[:, :, 256:260].unsqueeze(3)
                       .broadcast_to([128, gsz, H, HD]))
                o4 = u3[:, :, 0:256].rearrange("p b (h d) -> p b h d", h=H)
                nc.vector.tensor_tensor(o4, in0, in1, Alu.mult)

                for b in range(gsz):
                    t = w * tw + g0 + b
                    nc.tensor.matmul(psum_s[:], ohview[b],
                                     u4[:, b * 260:(b + 1) * 260],
                                     start=(t == w * tw),
                                     stop=(t == w * tw + tw - 1))

            # ---- window epilogue ----
            dsum = fxp.tile([128, 4], F32, tag="dsum")
            nc.vector.tensor_tensor(dsum[:], psum_s[:, 256:260], epsg[:],
                                    Alu.add)
            rec = fxp.tile([128, 4], F32, tag="rec")
            nc.vector.reciprocal(rec[:], dsum[:])
            t1 = fxp.tile([128, 256], F32, tag="t1")
            bv3 = bvrep[:].rearrange("p (h d) -> p h d", h=H)
            dt3 = (psum_s[:, 256:260].unsqueeze(2)
                   .broadcast_to([128, H, HD]))
            nc.vector.tensor_tensor(
                t1[:].rearrange("p (h d) -> p h d", h=H), bv3, dt3, Alu.mult)
            t2 = fxp.tile([128, 256], F32, tag="t2")
            nc.vector.tensor_tensor(t2[:], psum_s[:, 0:256], t1[:], Alu.add)
            outt = fxp.tile([128, 256], F32, tag="outt")
            rec3 = rec[:].unsqueeze(2).broadcast_to([128, H, HD])
            nc.vector.tensor_tensor(
                outt[:].rearrange("p (h d) -> p h d", h=H),
                t2[:].rearrange("p (h d) -> p h d", h=H), rec3, Alu.mult)
            nc.sync.dma_start(out_d[w * 128:(w + 1) * 128, :], outt[:])

    nc.compile()
    return nc


def kernel(x, batch, query, key_w, key_b, value_w, value_b):
    global LAST_RESULT
    from concourse.bass_utils import run_bass_kernel_spmd

    x = np.asarray(x, dtype=np.float32)
    batch = np.asarray(batch).astype(np.int64)
    query = np.asarray(query, dtype=np.float32)
    key_w = np.asarray(key_w, dtype=np.float32)
    key_b = np.asarray(key_b, dtype=np.float32)
    value_w = np.asarray(value_w, dtype=np.float32)
    value_b = np.asarray(value_b, dtype=np.float32)

    # ---- host-side planning ----
    counts = np.bincount(batch, minlength=B)
    cum = np.zeros(B + 1, np.int64)
    cum[1:] = np.cumsum(counts)
    nwin = NCORES * WPC
    wstart = cum[np.arange(nwin) * WSEG]
    wend = cum[(np.arange(nwin) + 1) * WSEG]
    tiles_w = (wend - wstart + 127) // 128
    tw = int(tiles_w.max())
    tw += tw % 2                      # keep P a multiple of CHUNK
    P = WPC * tw * 128

    # ---- shared constants ----
    wqf = np.zeros((256, 260), np.float32)
    wqf[:, 0:256] = value_w.T
    qt = (key_w.reshape(H, HD, DIM) * query[:, :, None]).sum(axis=1)  # [H,256]
    wqf[:, 256:260] = SCALE * qt.T
    wq = wqf.astype(np.float16)
    sc = SCALE * (query * key_b.reshape(H, HD)).sum(axis=1)           # [H]
    g = np.exp(sc).astype(np.float32)
    bvrep = np.broadcast_to(value_b, (128, 256)).astype(np.float32).copy()
    epsg = np.broadcast_to(EPS / g, (128, 4)).astype(np.float32).copy()

    # ---- per-core shards ----
    in_maps = []
    for c in range(NCORES):
        xTp = np.zeros((256, P), np.float16)
        ohp = np.zeros((128, P), np.float16)
        oh_t = ohp.reshape(128, P // 128, 128)        # [p, tile, j]
        for w in range(WPC):
            m = c * WPC + w
            ns, ne = int(wstart[m]), int(wend[m])
            L = ne - ns
            col0 = w * tw * 128
            xTp[:, col0:col0 + L] = x[ns:ne, :].T.astype(np.float16)
            j = (batch[ns:ne] - m * WSEG).astype(np.int64)
            node = np.arange(L) + col0
            oh_t[node % 128, node // 128, j] = np.float16(1.0)
        in_maps.append({"xT": xTp, "oh": ohp, "wq": wq,
                        "bvrep": bvrep, "epsg": epsg})

    if tw not in _cache:
        _cache[tw] = _build(tw)
    nc = _cache[tw]

    res = run_bass_kernel_spmd(nc, in_maps, core_ids=list(range(NCORES)),
                               trace=TRACE)
    LAST_RESULT = res
    return np.concatenate([r["out"] for r in res.results], axis=0)


# revision 10
# speedup vs baseline: 1.1753x; 1.1753x over previous
"""AttentionPooling segment-reduce kernel for 8 Trainium2 NeuronCores.

Math (reference):
    k = x @ key_w.T + key_b            # [N, 256] -> heads [N, 4, 64]
    v = x @ value_w.T + value_b
    attn   = einsum('hd,nhd->nh', query, k) * SCALE
    w      = exp(attn)
    wsum   = segment_sum(w)[batch]
    out[b] = segment_sum(w/(wsum+EPS) * v)

Algebraic restructuring (exact):
    attn[n,h] = qt[:,h] . x[n] + sc[h],  qt = SCALE*(key_w^T q per head),
                                         sc = SCALE*(q . key_b per head)
    w = exp(attn) = g[h]*wt[n,h],  wt = exp(qt . x),  g = exp(sc)
    v' = x @ value_w.T                 (bias deferred to segment level)
    St[b,f] = sum_{n in b} wt[n,h(f)] v'[n,f];  dt[b,h] = sum_{n in b} wt[n,h]
    out[b,f] = (St[b,f] + dt[b,h]*value_b[f]) / (dt[b,h] + EPS/g[h])

Device mapping: core c owns segments [c*512,(c+1)*512) split into 4 windows of
128 segments; window nodes padded to 128-multiples. Per 128-node tile:
 - PE: fused projection psum[nodes,260] = xT_tile.T @ [Wv^T | qt] (fp16 in,
   fp32 accum), then segment reduce psum_s[segs,260] += onehot.T @ u.
 - ACT: exp of the 4 attn columns (batched over a 3-tile group).
 - DVE: u[:,0:256] = psum[:,0:256] * wt (head-broadcast), one batched op/group.
One-hot node->segment matrices are precomputed on the host (exact 0/1 fp16)
and streamed alongside x^T, so no on-device index compute is needed.
Window epilogue (DVE): out = (St + dt*bv) / (dt + eps/g), DMA to the core's
output rows. Host pre-transposes x to [256, N] fp16 so the contraction dim
lands on SBUF partitions.
"""

from contextlib import ExitStack

import numpy as np

N = 262144
DIM = 256
H = 4
HD = 64
B = 4096
SCALE = HD ** (-0.5)
EPS = 1e-8

NCORES = 8
SEGS_PER_CORE = B // NCORES          # 512
WPC = 4                              # windows per core
WSEG = SEGS_PER_CORE // WPC          # 128 segments per window
GRP = 2                              # node-tiles per PSUM group
CHUNK = 2048                         # x columns per DMA chunk

TRACE = False                        # test harness can flip for profiling
LAST_RESULT = None

_cache = {}


def _build(tw: int):
    """Build + compile the SPMD program for tw node-tiles per window."""
    import concourse.tile as tile
    from concourse import bacc, mybir

    F32 = mybir.dt.float32
    F16 = mybir.dt.float16
    Alu = mybir.AluOpType
    Act = mybir.ActivationFunctionType

    P = WPC * tw * 128

    nc = bacc.Bacc("TRN2", target_bir_lowering=False, debug=False,
                   num_devices=NCORES)

    xT_d = nc.dram_tensor("xT", [256, P], F16, kind="ExternalInput").ap()
    oh_d = nc.dram_tensor("oh", [128, P], F16, kind="ExternalInput").ap()
    wq_d = nc.dram_tensor("wq", [256, 260], F16, kind="ExternalInput").ap()
    bvrep_d = nc.dram_tensor("bvrep", [128, 256], F32,
                             kind="ExternalInput").ap()
    epsg_d = nc.dram_tensor("epsg", [128, 4], F32, kind="ExternalInput").ap()
    out_d = nc.dram_tensor("out", [SEGS_PER_CORE, 256], F32,
                           kind="ExternalOutput").ap()

    with tile.TileContext(nc) as tc, ExitStack() as ctx:
        consts = ctx.enter_context(tc.tile_pool(name="consts", bufs=1))
        xin = ctx.enter_context(tc.tile_pool(name="xin", bufs=6))
        up = ctx.enter_context(tc.tile_pool(name="up", bufs=4))
        fxp = ctx.enter_context(tc.tile_pool(name="fxp", bufs=2))
        pp = ctx.enter_context(tc.tile_pool(name="pp", bufs=3, space="PSUM"))
        sp = ctx.enter_context(tc.tile_pool(name="sp", bufs=2, space="PSUM"))

        wq0 = consts.tile([128, 260], F16, tag="wq0")
        wq1 = consts.tile([128, 260], F16, tag="wq1")
        bvrep = consts.tile([128, 256], F32, tag="bvrep")
        epsg = consts.tile([128, 4], F32, tag="epsg")
        nc.sync.dma_start(wq0[:], wq_d[0:128, :])
        nc.sync.dma_start(wq1[:], wq_d[128:256, :])
        nc.sync.dma_start(bvrep[:], bvrep_d)
        nc.sync.dma_start(epsg[:], epsg_d)

        xt0 = xt1 = ohc = None
        for w in range(WPC):
            psum_s = sp.tile([128, 260], F32, tag="ps")
            for g0 in range(0, tw, GRP):
                gsz = min(GRP, tw - g0)
                psum4 = pp.tile([128, gsz * 512], F32, tag="pp")
                u4 = up.tile([128, gsz * 260], F16, tag="u4")
                ohview = []
                for b in range(gsz):
                    t = w * tw + g0 + b          # core-local tile index
                    col = t * 128
                    if col % CHUNK == 0:
                        cw = min(CHUNK, P - col)
                        xt0 = xin.tile([128, CHUNK], F16, tag="xt0")
                        xt1 = xin.tile([128, CHUNK], F16, tag="xt1")
                        ohc = xin.tile([128, CHUNK], F16, tag="ohc")
                        nc.sync.dma_start(xt0[:, 0:cw],
                                          xT_d[0:128, col:col + cw])
                        nc.sync.dma_start(xt1[:, 0:cw],
                                          xT_d[128:256, col:col + cw])
                        nc.sync.dma_start(ohc[:, 0:cw],
                                          oh_d[:, col:col + cw])
                    o = col % CHUNK
                    ps = psum4[:, b * 512:b * 512 + 260]
                    nc.tensor.matmul(ps, xt0[:, o:o + 128], wq0[:],
                                     start=True, stop=False)
                    nc.tensor.matmul(ps, xt1[:, o:o + 128], wq1[:],
                                     start=False, stop=True)
                    ohview.append(ohc[:, o:o + 128])

                p3 = psum4[:].rearrange("p (b c) -> p b c", c=512)
                u3 = u4[:].rearrange("p (b c) -> p b c", c=260)
                nc.scalar.activation(u3[:, :, 256:260], p3[:, :, 256:260],
                                     Act.Exp)
                in0 = p3[:, :, 0:256].rearrange("p b (h d) -> p b h d", h=H)
                in1 = (u3[:, :, 256:260].unsqueeze(3)
                       .broadcast_to([128, gsz, H, HD]))
                o4 = u3[:, :, 0:256].rearrange("p b (h d) -> p b h d", h=H)
                nc.vector.tensor_tensor(o4, in0, in1, Alu.mult)

                for b in range(gsz):
                    t = w * tw + g0 + b
                    nc.tensor.matmul(psum_s[:], ohview[b],
                                     u4[:, b * 260:(b + 1) * 260],
                                     start=(t == w * tw),
                                     stop=(t == w * tw + tw - 1))

            # ---- window epilogue ----
            dsum = fxp.tile([128, 4], F32, tag="dsum")
            nc.vector.tensor_tensor(dsum[:], psum_s[:, 256:260], epsg[:],
                                    Alu.add)
            rec = fxp.tile([128, 4], F32, tag="rec")
            nc.vector.reciprocal(rec[:], dsum[:])
            t1 = fxp.tile([128, 256], F32, tag="t1")
            bv3 = bvrep[:].rearrange("p (h d) -> p h d", h=H)
            dt3 = (psum_s[:, 256:260].unsqueeze(2)
                   .broadcast_to([128, H, HD]))
            nc.vector.tensor_tensor(
                t1[:].rearrange("p (h d) -> p h d", h=H), bv3, dt3, Alu.mult)
            t2 = fxp.tile([128, 256], F32, tag="t2")
            nc.vector.tensor_tensor(t2[:], psum_s[:, 0:256], t1[:], Alu.add)
            outt = fxp.tile([128, 256], F32, tag="outt")
            rec3 = rec[:].unsqueeze(2).broadcast_to([128, H, HD])
            nc.vector.tensor_tensor(
                outt[:].rearrange("p (h d) -> p h d", h=H),
                t2[:].rearrange("p (h d) -> p h d", h=H), rec3, Alu.mult)
            nc.sync.dma_start(out_d[w * 128:(w + 1) * 128, :], outt[:])

    nc.compile()
    return nc


def kernel(x, batch, query, key_w, key_b, value_w, value_b):
    global LAST_RESULT
    from concourse.bass_utils import run_bass_kernel_spmd

    x = np.asarray(x, dtype=np.float32)
    batch = np.asarray(batch).astype(np.int64)
    query = np.asarray(query, dtype=np.float32)
    key_w = np.asarray(key_w, dtype=np.float32)
    key_b = np.asarray(key_b, dtype=np.float32)
    value_w = np.asarray(value_w, dtype=np.float32)
    value_b = np.asarray(value_b, dtype=np.float32)

    # ---- host-side planning ----
    counts = np.bincount(batch, minlength=B)
    cum = np.zeros(B + 1, np.int64)
    cum[1:] = np.cumsum(counts)
    nwin = NCORES * WPC
    wstart = cum[np.arange(nwin) * WSEG]
    wend = cum[(np.arange(nwin) + 1) * WSEG]
    tiles_w = (wend - wstart + 127) // 128
    tw = int(tiles_w.max())
    tw += tw % 2                      # keep P a multiple of CHUNK
    P = WPC * tw * 128

    # ---- shared constants ----
    wqf = np.zeros((256, 260), np.float32)
    wqf[:, 0:256] = value_w.T
    qt = (key_w.reshape(H, HD, DIM) * query[:, :, None]).sum(axis=1)  # [H,256]
    wqf[:, 256:260] = SCALE * qt.T
    wq = wqf.astype(np.float16)
    sc = SCALE * (query * key_b.reshape(H, HD)).sum(axis=1)           # [H]
    g = np.exp(sc).astype(np.float32)
    bvrep = np.broadcast_to(value_b, (128, 256)).astype(np.float32).copy()
    epsg = np.broadcast_to(EPS / g, (128, 4)).astype(np.float32).copy()

    # ---- per-core shards ----
    in_maps = []
    for c in range(NCORES):
        xTp = np.zeros((256, P), np.float16)
        ohp = np.zeros((128, P), np.float16)
        oh_t = ohp.reshape(128, P // 128, 128)        # [p, tile, j]
        for w in range(WPC):
            m = c * WPC + w
            ns, ne = int(wstart[m]), int(wend[m])
            L = ne - ns
            col0 = w * tw * 128
            xTp[:, col0:col0 + L] = x[ns:ne, :].T.astype(np.float16)
            j = (batch[ns:ne] - m * WSEG).astype(np.int64)
            node = np.arange(L) + col0
            oh_t[node % 128, node // 128, j] = np.float16(1.0)
        in_maps.append({"xT": xTp, "oh": ohp, "wq": wq,
                        "bvrep": bvrep, "epsg": epsg})

    if tw not in _cache:
        _cache[tw] = _build(tw)
    nc = _cache[tw]

    res = run_bass_kernel_spmd(nc, in_maps, core_ids=list(range(NCORES)),
                               trace=TRACE)
    LAST_RESULT = res
    return np.concatenate([r["out"] for r in res.results], axis=0)


# revision 11
# speedup vs baseline: 1.1933x; 1.0153x over previous
"""AttentionPooling segment-reduce kernel for 8 Trainium2 NeuronCores.

Math (reference):
    k = x @ key_w.T + key_b            # [N, 256] -> heads [N, 4, 64]
    v = x @ value_w.T + value_b
    attn   = einsum('hd,nhd->nh', query, k) * SCALE
    w      = exp(attn)
    wsum   = segment_sum(w)[batch]
    out[b] = segment_sum(w/(wsum+EPS) * v)

Algebraic restructuring (exact):
    attn[n,h] = qt[:,h] . x[n] + sc[h],  qt = SCALE*(key_w^T q per head),
                                         sc = SCALE*(q . key_b per head)
    w = exp(attn) = g[h]*wt[n,h],  wt = exp(qt . x),  g = exp(sc)
    v' = x @ value_w.T                 (bias deferred to segment level)
    St[b,f] = sum_{n in b} wt[n,h(f)] v'[n,f];  dt[b,h] = sum_{n in b} wt[n,h]
    out[b,f] = (St[b,f] + dt[b,h]*value_b[f]) / (dt[b,h] + EPS/g[h])

Device mapping: core c owns segments [c*512,(c+1)*512) split into 4 windows of
128 segments; window nodes padded to 128-multiples. Per 128-node tile:
 - PE: fused projection psum[nodes,260] = xT_tile.T @ [Wv^T | qt] (fp16 in,
   fp32 accum), then segment reduce psum_s[segs,260] += onehot.T @ u.
 - ACT: exp of the 4 attn columns (batched over a 3-tile group).
 - DVE: u[:,0:256] = psum[:,0:256] * wt (head-broadcast), one batched op/group.
One-hot node->segment matrices are precomputed on the host (exact 0/1 fp16)
and streamed alongside x^T, so no on-device index compute is needed.
Window epilogue (DVE): out = (St + dt*bv) / (dt + eps/g), DMA to the core's
output rows. Host pre-transposes x to [256, N] fp16 so the contraction dim
lands on SBUF partitions.
"""

from contextlib import ExitStack

import numpy as np

N = 262144
DIM = 256
H = 4
HD = 64
B = 4096
SCALE = HD ** (-0.5)
EPS = 1e-8

NCORES = 8
SEGS_PER_CORE = B // NCORES          # 512
WPC = 4                              # windows per core
WSEG = SEGS_PER_CORE // WPC          # 128 segments per window
GRP = 2                              # node-tiles per PSUM group
CHUNK = 1024                         # x columns per DMA chunk

TRACE = False                        # test harness can flip for profiling
LAST_RESULT = None

_cache = {}


def _build(tw: int):
    """Build + compile the SPMD program for tw node-tiles per window."""
    import concourse.tile as tile
    from concourse import bacc, mybir

    F32 = mybir.dt.float32
    F16 = mybir.dt.float16
    Alu = mybir.AluOpType
    Act = mybir.ActivationFunctionType

    P = WPC * tw * 128

    nc = bacc.Bacc("TRN2", target_bir_lowering=False, debug=False,
                   num_devices=NCORES)

    xT_d = nc.dram_tensor("xT", [256, P], F16, kind="ExternalInput").ap()
    oh_d = nc.dram_tensor("oh", [128, P], F16, kind="ExternalInput").ap()
    wq_d = nc.dram_tensor("wq", [256, 260], F16, kind="ExternalInput").ap()
    bvrep_d = nc.dram_tensor("bvrep", [128, 256], F32,
                             kind="ExternalInput").ap()
    epsg_d = nc.dram_tensor("epsg", [128, 4], F32, kind="ExternalInput").ap()
    out_d = nc.dram_tensor("out", [SEGS_PER_CORE, 256], F32,
                           kind="ExternalOutput").ap()

    with tile.TileContext(nc) as tc, ExitStack() as ctx:
        consts = ctx.enter_context(tc.tile_pool(name="consts", bufs=1))
        xin = ctx.enter_context(tc.tile_pool(name="xin", bufs=6))
        up = ctx.enter_context(tc.tile_pool(name="up", bufs=4))
        fxp = ctx.enter_context(tc.tile_pool(name="fxp", bufs=2))
        pp = ctx.enter_context(tc.tile_pool(name="pp", bufs=3, space="PSUM"))
        sp = ctx.enter_context(tc.tile_pool(name="sp", bufs=2, space="PSUM"))

        wq0 = consts.tile([128, 260], F16, tag="wq0")
        wq1 = consts.tile([128, 260], F16, tag="wq1")
        bvrep = consts.tile([128, 256], F32, tag="bvrep")
        epsg = consts.tile([128, 4], F32, tag="epsg")
        nc.sync.dma_start(wq0[:], wq_d[0:128, :])
        nc.sync.dma_start(wq1[:], wq_d[128:256, :])
        nc.sync.dma_start(bvrep[:], bvrep_d)
        nc.sync.dma_start(epsg[:], epsg_d)

        xt0 = xt1 = ohc = None
        for w in range(WPC):
            psum_s = sp.tile([128, 260], F32, tag="ps")
            for g0 in range(0, tw, GRP):
                gsz = min(GRP, tw - g0)
                psum4 = pp.tile([128, gsz * 512], F32, tag="pp")
                u4 = up.tile([128, gsz * 260], F16, tag="u4")
                ohview = []
                for b in range(gsz):
                    t = w * tw + g0 + b          # core-local tile index
                    col = t * 128
                    if col % CHUNK == 0:
                        cw = min(CHUNK, P - col)
                        xt0 = xin.tile([128, CHUNK], F16, tag="xt0")
                        xt1 = xin.tile([128, CHUNK], F16, tag="xt1")
                        ohc = xin.tile([128, CHUNK], F16, tag="ohc")
                        nc.sync.dma_start(xt0[:, 0:cw],
                                          xT_d[0:128, col:col + cw])
                        nc.sync.dma_start(xt1[:, 0:cw],
                                          xT_d[128:256, col:col + cw])
                        nc.sync.dma_start(ohc[:, 0:cw],
                                          oh_d[:, col:col + cw])
                    o = col % CHUNK
                    ps = psum4[:, b * 512:b * 512 + 260]
                    nc.tensor.matmul(ps, xt0[:, o:o + 128], wq0[:],
                                     start=True, stop=False)
                    nc.tensor.matmul(ps, xt1[:, o:o + 128], wq1[:],
                                     start=False, stop=True)
                    ohview.append(ohc[:, o:o + 128])

                p3 = psum4[:].rearrange("p (b c) -> p b c", c=512)
                u3 = u4[:].rearrange("p (b c) -> p b c", c=260)
                nc.scalar.activation(u3[:, :, 256:260], p3[:, :, 256:260],
                                     Act.Exp)
                in0 = p3[:, :, 0:256].rearrange("p b (h d) -> p b h d", h=H)
                in1 = (u3[:, :, 256:260].unsqueeze(3)
                       .broadcast_to([128, gsz, H, HD]))
                o4 = u3[:, :, 0:256].rearrange("p b (h d) -> p b h d", h=H)
                nc.vector.tensor_tensor(o4, in0, in1, Alu.mult)

                for b in range(gsz):
                    t = w * tw + g0 + b
                    nc.tensor.matmul(psum_s[:], ohview[b],
                                     u4[:, b * 260:(b + 1) * 260],
                                     start=(t == w * tw),
                                     stop=(t == w * tw + tw - 1))

            # ---- window epilogue ----
            dsum = fxp.tile([128, 4], F32, tag="dsum")
            nc.vector.tensor_tensor(dsum[:], psum_s[:, 256:260], epsg[:],
                                    Alu.add)
            rec = fxp.tile([128, 4], F32, tag="rec")
            nc.vector.reciprocal(rec[:], dsum[:])
            t1 = fxp.tile([128, 256], F32, tag="t1")
            bv3 = bvrep[:].rearrange("p (h d) -> p h d", h=H)
            dt3 = (psum_s[:, 256:260].unsqueeze(2)
                   .broadcast_to([128, H, HD]))
            nc.vector.tensor_tensor(
                t1[:].rearrange("p (h d) -> p h d", h=H), bv3, dt3, Alu.mult)
            t2 = fxp.tile([128, 256], F32, tag="t2")
            nc.vector.tensor_tensor(t2[:], psum_s[:, 0:256], t1[:], Alu.add)
            outt = fxp.tile([128, 256], F32, tag="outt")
            rec3 = rec[:].unsqueeze(2).broadcast_to([128, H, HD])
            nc.vector.tensor_tensor(
                outt[:].rearrange("p (h d) -> p h d", h=H),
                t2[:].rearrange("p (h d) -> p h d", h=H), rec3, Alu.mult)
            nc.sync.dma_start(out_d[w * 128:(w + 1) * 128, :], outt[:])

    nc.compile()
    return nc


def kernel(x, batch, query, key_w, key_b, value_w, value_b):
    global LAST_RESULT
    from concourse.bass_utils import run_bass_kernel_spmd

    x = np.asarray(x, dtype=np.float32)
    batch = np.asarray(batch).astype(np.int64)
    query = np.asarray(query, dtype=np.float32)
    key_w = np.asarray(key_w, dtype=np.float32)
    key_b = np.asarray(key_b, dtype=np.float32)
    value_w = np.asarray(value_w, dtype=np.float32)
    value_b = np.asarray(value_b, dtype=np.float32)

    # ---- host-side planning ----
    counts = np.bincount(batch, minlength=B)
    cum = np.zeros(B + 1, np.int64)
    cum[1:] = np.cumsum(counts)
    nwin = NCORES * WPC
    wstart = cum[np.arange(nwin) * WSEG]
    wend = cum[(np.arange(nwin) + 1) * WSEG]
    tiles_w = (wend - wstart + 127) // 128
    tw = int(tiles_w.max())
    tw += tw % 2                      # keep P a multiple of CHUNK
    P = WPC * tw * 128

    # ---- shared constants ----
    wqf = np.zeros((256, 260), np.float32)
    wqf[:, 0:256] = value_w.T
    qt = (key_w.reshape(H, HD, DIM) * query[:, :, None]).sum(axis=1)  # [H,256]
    wqf[:, 256:260] = SCALE * qt.T
    wq = wqf.astype(np.float16)
    sc = SCALE * (query * key_b.reshape(H, HD)).sum(axis=1)           # [H]
    g = np.exp(sc).astype(np.float32)
    bvrep = np.broadcast_to(value_b, (128, 256)).astype(np.float32).copy()
    epsg = np.broadcast_to(EPS / g, (128, 4)).astype(np.float32).copy()

    # ---- per-core shards ----
    in_maps = []
    for c in range(NCORES):
        xTp = np.zeros((256, P), np.float16)
        ohp = np.zeros((128, P), np.float16)
        oh_t = ohp.reshape(128, P // 128, 128)        # [p, tile, j]
        for w in range(WPC):
            m = c * WPC + w
            ns, ne = int(wstart[m]), int(wend[m])
            L = ne - ns
            col0 = w * tw * 128
            xTp[:, col0:col0 + L] = x[ns:ne, :].T.astype(np.float16)
            j = (batch[ns:ne] - m * WSEG).astype(np.int64)
            node = np.arange(L) + col0
            oh_t[node % 128, node // 128, j] = np.float16(1.0)
        in_maps.append({"xT": xTp, "oh": ohp, "wq": wq,
                        "bvrep": bvrep, "epsg": epsg})

    if tw not in _cache:
        _cache[tw] = _build(tw)
    nc = _cache[tw]

    res = run_bass_kernel_spmd(nc, in_maps, core_ids=list(range(NCORES)),
                               trace=TRACE)
    LAST_RESULT = res
    return np.concatenate([r["out"] for r in res.results], axis=0)
